# revision 1
# baseline (speedup 1.0000x reference)
"""GAT layer (B=4, N=2048, D=256, H=4) on 8 trn2 NeuronCores.

Sharding: core c -> (b = c//2, i-half = c%2).  Each core computes
out[b, ihalf*1024:(ihalf+1)*1024, :]; h is computed on-device from the full
x[b] (passed pre-transposed as x[b].T, bf16).

Math: with z = s_src[i] + s_dst[j], the reference computes
    alpha = softmax_j(mask(leaky_relu(z)));  out = alpha @ h_head.
Softmax is shift-invariant per destination row i, so we use shifted
unnormalized weights (same alpha):
    P[i,j] = adj[i,j] * exp(leaky_relu(z) - s_src[i])
           = adj[i,j] * max(F1[j], E2[i] * J[j])
with F1 = exp(s_dst), J = exp(0.2*s_dst), E2 = exp(-0.8*s_src)
(z>=0 branch gives exp(s_dst)=F1; z<0 branch gives exp(0.2z-s_src)=E2*J).
Row sums come from an appended ones-column in the aggregation matmul
stationary:  psoT = [h_head | 1].T @ P^T  ->  out = psoT[:64] / psoT[64].
"""

import sys

for _p in ("/opt/trn_rl_repo", "/root/.axon_site/_ro/trn_rl_repo"):
    if _p not in sys.path:
        sys.path.insert(0, _p)

import ml_dtypes
import numpy as np

import concourse.bass as bass
import concourse.mybir as mybir
from concourse import masks, tile
from concourse.bass_utils import run_bass_kernel_spmd
from concourse.vector_clock import ScopedClock

F32 = mybir.dt.float32
BF16 = mybir.dt.bfloat16
AF = mybir.ActivationFunctionType
ALU = mybir.AluOpType

B, N, D, H, HD = 4, 2048, 256, 4, 64
NEG_SLOPE = 0.2
P = 128
NI = N // 2          # i-rows per core (1024)
NT = N // P          # 16 j/n tiles
KT = D // P          # 2 k tiles
JT = NT
ISUB = NI // P       # 8
NCORES = 8
WC = D + 3 * H       # 268 aug cols: [W.T | WtAdst | 0.2*WtAdst | WtAsrc]
HP1 = HD + 1         # 65
HPW = H * HP1        # 260 hplus cols per j-tile

# NOTE: offloading P^T elementwise work off the DVE is a dead end on trn2:
# gpsimd's software tensor_scalar is ~15us per [128,1024] op AND its SBUF
# traffic starves the DVE (shared read ports) — measured 2.8x regression;
# the ACT-assisted w=Relu(...) + scalar_tensor_tensor path is a wash because
# scalar_tensor_tensor runs at 1x (1276ns) vs tensor_scalar 4x + tensor_tensor
# 2x (535+682ns merged) — measured, not modeled.
ACT_JT = ()


def _patch_tile_drain():
    """walrus rejects >1 sem wait on one instruction in this toolchain; split
    the TileContext tail drain's waits across consecutive SP drains."""
    if getattr(tile.TileContext, "_drain_patched", False):
        return

    def _drain_and_barrier(self, tick_clock, wait_clock):
        nc = self.nc
        drain_inst = nc.sync.drain()
        wait_clock.add_sem_waits(
            drain_inst.ins, ScopedClock({None: tick_clock.global_clock})
        )
        si = drain_inst.ins.sync_info
        waits = list(si.on_wait) if (si and si.on_wait) else []
        if len(waits) > 1:
            ups = list(si.on_update) if (si and si.on_update) else []
            drain_inst.ins.sync_info = mybir.SyncInfo(on_wait=waits[:1], on_update=ups)
            for i in range(1, len(waits)):
                extra = nc.sync.drain()
                extra.ins.sync_info = mybir.SyncInfo(
                    on_wait=waits[i : i + 1], on_update=[]
                )
        nc.all_engine_barrier()
        assert self.sems is not None
        popped = nc._tile_sem_poison_stack.pop()
        assert popped is self._sem_poison
        nc.clear_and_free_semaphores(list(self.sems.allocated().values()))
        nc.all_engine_barrier()

    tile.TileContext._drain_and_barrier = _drain_and_barrier
    tile.TileContext._drain_patched = True


def _split_waits(nc, maxw=1):
    """Hoist excess sem waits onto same-engine EventSemaphore carriers placed
    just before the instruction (same engine + program order => equivalent)."""
    n_split = 0
    for f in nc.m.functions:
        for bb in f.blocks:
            insts = list(bb.instructions)
            out = []
            changed = False
            for inst in insts:
                si = inst.sync_info
                waits = list(si.on_wait) if (si and si.on_wait) else []
                if len(waits) > maxw and inst.engine is not None:
                    changed = True
                    extra, keep = waits[:-maxw], waits[-maxw:]
                    for k in range(0, len(extra), maxw):
                        d = mybir.InstEventSemaphore(
                            name=f"{inst.name}-wsplit{k}", ins=[], outs=[]
                        )
                        d.engine = inst.engine
                        d.sync_info = mybir.SyncInfo(
                            on_wait=extra[k : k + maxw], on_update=[]
                        )
                        out.append(d)
                        n_split += 1
                    ups = list(si.on_update) if (si and si.on_update) else []
                    inst.sync_info = mybir.SyncInfo(on_wait=keep, on_update=ups)
                out.append(inst)
            if changed:
                bb.instructions = out
    return n_split


def build_nc():
    _patch_tile_drain()
    nc = bass.Bass("TRN2", target_bir_lowering=False, debug=False)

    xbt = nc.dram_tensor("xbt", [D, N], BF16, kind="ExternalInput")    # x[b].T
    xit = nc.dram_tensor("xit", [D, NI], BF16, kind="ExternalInput")   # xi.T
    wta = nc.dram_tensor("wta", [D, WC], BF16, kind="ExternalInput")
    adjtb = nc.dram_tensor("adjtb", [N, NI], BF16, kind="ExternalInput")
    selm = nc.dram_tensor("selm", [H, H * P], BF16, kind="ExternalInput")
    outs = nc.dram_tensor("outs", [NI, D], F32, kind="ExternalOutput")

    with tile.TileContext(nc) as tc:
        with (
            tc.tile_pool(name="const", bufs=1) as constp,
            tc.tile_pool(name="big", bufs=1) as bigp,
            tc.tile_pool(name="rows", bufs=1) as rowsp,
            tc.tile_pool(name="jf", bufs=16) as jfp,
            tc.tile_pool(name="adjt", bufs=17) as adjtp,
            tc.tile_pool(name="vwork", bufs=3) as vp,
            tc.tile_pool(name="ptwork", bufs=5) as ptp,
            tc.tile_pool(name="sot", bufs=3) as sotp,
            tc.tile_pool(name="small", bufs=6) as smallp,
            tc.tile_pool(name="psall", bufs=1, space="PSUM") as psall,
        ):
            # transient psum tiles round-robin banks 4-7; psoT/ps2 use banks 0-3
            ps_ctr = [0]

            def ps_tile(shape, name, tag=None):
                if tag is None:
                    tag = f"bank{4 + ps_ctr[0] % 4}"
                    ps_ctr[0] += 1
                return psall.tile(shape, F32, tag=tag, name=name)

            pe_prev = [None]

            def pe(bi):
                # pin PE stream order: PSUM accumulation groups must stay
                # contiguous on PE (interleaving corrupts accumulation on HW)
                if pe_prev[0] is not None:
                    tile.add_dep_helper(bi.ins, pe_prev[0], reason="pe-order")
                pe_prev[0] = bi.ins
                return bi

            ident = constp.tile([P, P], F32, tag="ident")
            masks.make_identity(nc, ident[:])

            wta_sb = [
                constp.tile([P, WC], BF16, tag=f"wta{kt}", name=f"wta_sb{kt}")
                for kt in range(KT)
            ]
            sel_sb = constp.tile([H, H * P], BF16, tag="selm")
            sels = [sel_sb[:, h * P : (h + 1) * P] for h in range(H)]
            for kt in range(KT):
                nc.sync.dma_start(wta_sb[kt][:], wta[kt * P : (kt + 1) * P, :])

            # ---- x^T loads (bf16, no conversion passes), chunked so the
            # s_src chain (pss -> er4 -> e2rep) starts as early as possible
            xit_sb = bigp.tile([P, KT * NI], BF16, tag="xit")
            xt_sb = bigp.tile([P, KT * N], BF16, tag="xt")
            for c in range(2):
                for kt in range(KT):
                    nc.sync.dma_start(
                        xit_sb[:, kt * NI + c * 512 : kt * NI + (c + 1) * 512],
                        xit[kt * P : (kt + 1) * P, c * 512 : (c + 1) * 512],
                    )
                if c == 0:
                    for kt in range(KT):
                        nc.sync.dma_start(
                            xt_sb[:, kt * N : kt * N + 512],
                            xbt[kt * P : (kt + 1) * P, 0:512],
                        )
                    nc.sync.dma_start(sel_sb[:], selm[:])
            # hplus memset early: gates the per-nt strided copies
            hplus = bigp.tile([P, NT * HPW], BF16, tag="hplus")
            nc.gpsimd.memset(hplus[:], 1.0)
            hp4 = hplus[:].rearrange("p (t h c) -> p t h c", t=NT, h=H)

            adjts = [
                adjtp.tile([P, NI], BF16, tag="adjt", name=f"adjt_{jt}")
                for jt in range(JT)
            ]
            for c in range(1, 4):
                for kt in range(KT):
                    nc.sync.dma_start(
                        xt_sb[:, kt * N + c * 512 : kt * N + (c + 1) * 512],
                        xbt[kt * P : (kt + 1) * P, c * 512 : (c + 1) * 512],
                    )
                nc.sync.dma_start(
                    adjts[c - 1][:], adjtb[(c - 1) * P : c * P, :]
                )
            for jt in range(3, JT):
                nc.gpsimd.dma_start(adjts[jt][:], adjtb[jt * P : (jt + 1) * P, :])

            # ---- s_srcT (all heads) -> E2 rows [4, NI] -> selector bcast ----
            er4 = rowsp.tile([H, NI], BF16, tag="er4")
            for c in range(NI // 512):
                pss = ps_tile([H, 512], f"pss_{c}")
                for kt in range(KT):
                    pe(nc.tensor.matmul(
                        pss[:],
                        wta_sb[kt][:, D + 2 * H : D + 3 * H],
                        xit_sb[:, kt * NI + c * 512 : kt * NI + (c + 1) * 512],
                        start=(kt == 0),
                        stop=(kt == KT - 1),
                    ))
                nc.scalar.activation(
                    er4[:, c * 512 : (c + 1) * 512],
                    pss[:],
                    AF.Exp,
                    scale=-(1.0 - NEG_SLOPE),
                )
            e2rep = bigp.tile([P, H * NI], BF16, tag="e2rep")

            def emit_e2rep(h):
                for c in range(NI // 512):
                    psb = ps_tile([P, 512], f"psb_{h}_{c}")
                    pe(nc.tensor.matmul(
                        psb[:], sels[h], er4[0:H, c * 512 : (c + 1) * 512]
                    ))
                    nc.scalar.activation(
                        e2rep[:, h * NI + c * 512 : h * NI + (c + 1) * 512],
                        psb[:],
                        AF.Copy,
                    )

            emit_e2rep(0)

            # ---- h_aug = x @ wta (bf16); JF = [F1|J]; hplus strided copy ----
            jf_tiles = []
            njf_tiles = {}
            for nt in range(NT):
                if nt == 1:
                    emit_e2rep(1)
                if nt == 3:
                    emit_e2rep(2)
                if nt == 5:
                    emit_e2rep(3)
                psh = ps_tile([P, WC], f"psh_{nt}")
                for kt in range(KT):
                    pe(nc.tensor.matmul(
                        psh[:],
                        xt_sb[:, kt * N + nt * P : kt * N + (nt + 1) * P],
                        wta_sb[kt][:],
                        start=(kt == 0),
                        stop=(kt == KT - 1),
                    ))
                jf = jfp.tile([P, 2 * H], F32, tag="jf", name=f"jf_{nt}")
                nc.scalar.activation(jf[:], psh[:, D : D + 2 * H], AF.Exp)
                jf_tiles.append(jf)
                nc.scalar.activation(
                    hp4[:, nt, :, 0:HD],
                    psh[:, 0:D].rearrange("p (h c) -> p h c", h=H),
                    AF.Copy,
                )

            # ---- main: P^T construction (DVE) + aggregation + epilogue ----
            ost = bigp.tile([P, ISUB * D], F32, tag="ost")
            ost8 = ost[:].rearrange("p (s c) -> p s c", s=ISUB)

            # epilogue for one (h, half) combo, deferred so it can be injected
            # into the NEXT pair's production streams without stalling them
            def emit_epilogue(pair, h01, half, psoT):
                h = 2 * pair + h01
                soT = sotp.tile([HP1, 512], F32, tag="soT", name=f"soT_{h}_{half}")
                nc.scalar.activation(soT[:], psoT[:], AF.Copy)
                ps2 = psall.tile(
                    [P, H * HP1], F32, tag=f"bank{h01 * 2 + half}",
                    name=f"ps2_{h}_{half}",
                )
                for q in range(4):
                    pe(nc.tensor.transpose(
                        ps2[:, q * HP1 : (q + 1) * HP1],
                        soT[:, q * P : (q + 1) * P],
                        ident[0:HP1, 0:HP1],
                    ))
                ps2q = ps2[:].rearrange("p (q c) -> p q c", q=4)
                rec4 = smallp.tile([P, 4], F32, tag="rec", name=f"rec_{h}_{half}")
                nc.vector.reciprocal(rec4[:], ps2q[:, :, HD])
                # scaled copies on ACT (idle in the main phase) to keep the
                # normalize off the DVE critical path
                for q in range(4):
                    nc.scalar.activation(
                        ost8[:, half * 4 + q, h * HD : (h + 1) * HD],
                        ps2q[:, q, 0:HD],
                        AF.Copy,
                        scale=rec4[:, q : q + 1],
                    )

            pending_epi = []
            for pair in range(2):
                # previous pair's epilogues first: with jt-major agg all its
                # groups ended together, and the psum tag chain
                # psoT(prev) -> soT -> ps2(prev) -> epi -> psoT(this) must
                # match PE program order (transposes before this pair's aggs)
                while pending_epi:
                    emit_epilogue(*pending_epi.pop(0))
                psoTs = {}
                for h01 in range(2):
                    for half in range(2):
                        psoTs[(h01, half)] = psall.tile(
                            [HP1, 512], F32, tag=f"bank{h01 * 2 + half}",
                            name=f"psoT_{2 * pair + h01}_{half}",
                        )
                for ji, jt in enumerate(range(JT)):
                    # v = max(e2rep*J, F1) per head; pt = v*adj (merged 2-head)
                    v2 = vp.tile([P, 2 * NI], BF16, tag="v", name=f"v_{pair}_{jt}")
                    for h01 in range(2):
                        h = 2 * pair + h01
                        nc.vector.tensor_scalar(
                            v2[:, h01 * NI : (h01 + 1) * NI],
                            e2rep[:, h * NI : (h + 1) * NI],
                            jf_tiles[jt][:, H + h : H + h + 1],
                            jf_tiles[jt][:, h : h + 1],
                            ALU.mult,
                            ALU.max,
                        )
                    pt2 = ptp.tile([P, 2 * NI], BF16, tag="pt", name=f"pt_{pair}_{jt}")
                    nc.vector.tensor_tensor(
                        pt2[:].rearrange("p (g c) -> p g c", g=2),
                        v2[:].rearrange("p (g c) -> p g c", g=2),
                        adjts[jt][:].unsqueeze(1).broadcast_to([P, 2, NI]),
                        ALU.mult,
                    )
                    # jt-major aggregation: 4 interleaved accumulation groups
                    # (verified on HW: per-cell has_written bits make
                    # interleaved groups on different banks safe)
                    for h01 in range(2):
                        h = 2 * pair + h01
                        for half in range(2):
                            pe(nc.tensor.matmul(
                                psoTs[(h01, half)][:],
                                hplus[:, jt * HPW + h * HP1 : jt * HPW + (h + 1) * HP1],
                                pt2[:, h01 * NI + half * 512 : h01 * NI + (half + 1) * 512],
                                start=(ji == 0),
                                stop=(ji == JT - 1),
                                skip_group_check=True,
                            ))
                for half in range(2):
                    for h01 in range(2):
                        pending_epi.append((pair, h01, half, psoTs[(h01, half)]))

            # final epilogues half-major; stream each half's output as soon as
            # its last head lands, splitting DMA issue across two queues
            emit_epilogue(*pending_epi.pop(0))
            emit_epilogue(*pending_epi.pop(0))
            for isub in range(4):
                eng = nc.sync if isub % 2 == 0 else nc.gpsimd
                eng.dma_start(
                    outs[isub * P : (isub + 1) * P, :],
                    ost[:, isub * D : (isub + 1) * D],
                )
            emit_epilogue(*pending_epi.pop(0))
            emit_epilogue(*pending_epi.pop(0))
            for isub in range(4, ISUB):
                eng = nc.sync if isub % 2 == 0 else nc.gpsimd
                eng.dma_start(
                    outs[isub * P : (isub + 1) * P, :],
                    ost[:, isub * D : (isub + 1) * D],
                )

    _split_waits(nc)
    nc.finalize()
    return nc


_NC_CACHE = None


def _get_nc():
    global _NC_CACHE
    if _NC_CACHE is None:
        _NC_CACHE = build_nc()
    return _NC_CACHE


def make_in_maps(x, adj, W, a_src, a_dst):
    x = np.ascontiguousarray(x, dtype=np.float32)
    W = np.ascontiguousarray(W, dtype=np.float32)
    a_src = np.ascontiguousarray(a_src, dtype=np.float32)
    a_dst = np.ascontiguousarray(a_dst, dtype=np.float32)

    A_src = np.zeros((D, H), np.float32)
    A_dst = np.zeros((D, H), np.float32)
    for h in range(H):
        A_src[h * HD : (h + 1) * HD, h] = a_src[h]
        A_dst[h * HD : (h + 1) * HD, h] = a_dst[h]
    Wt = W.T.astype(np.float32)
    wd = Wt @ A_dst
    wta = np.concatenate(
        [Wt, wd, NEG_SLOPE * wd, Wt @ A_src], axis=1
    ).astype(ml_dtypes.bfloat16)

    selm = np.zeros((H, H * P), ml_dtypes.bfloat16)
    for h in range(H):
        selm[h, h * P : (h + 1) * P] = 1.0
    in_maps = []
    adjT_cache = {}
    for c in range(NCORES):
        b, ihalf = c // 2, c % 2
        ilo = ihalf * NI
        if b not in adjT_cache:
            adjT_cache[b] = adj[b].astype(ml_dtypes.bfloat16).T
        in_maps.append(
            {
                "xbt": np.ascontiguousarray(x[b].T.astype(ml_dtypes.bfloat16)),
                "xit": np.ascontiguousarray(
                    x[b, ilo : ilo + NI, :].T.astype(ml_dtypes.bfloat16)
                ),
                "wta": np.ascontiguousarray(wta),
                "adjtb": np.ascontiguousarray(adjT_cache[b][:, ilo : ilo + NI]),
                "selm": selm,
            }
        )
    return in_maps


def kernel(x, adj, W, a_src, a_dst):
    in_maps = make_in_maps(x, adj, W, a_src, a_dst)
    nc = _get_nc()
    res = run_bass_kernel_spmd(nc, in_maps, list(range(NCORES)))

    out = np.empty((B, N, D), np.float32)
    for c in range(NCORES):
        b, ihalf = c // 2, c % 2
        ilo = ihalf * NI
        out[b, ilo : ilo + NI, :] = res.results[c]["outs"]
    return out

